# revision 8
# baseline (speedup 1.0000x reference)
"""CentroidInstanceLoss on 8 Trainium2 NeuronCores (Bass/Tile).

Subbatch-parallel: core c processes exactly the points of subbatch c
(S=8 == NCORES), padded to a common tile count T_pad. All centroid
segment-sums are then core-LOCAL (64 labels per core): no collectives,
no cross-core barrier. Host does only integer bookkeeping (counts, M,
pull weights, the subbatch partition/pad permutation) and the final
O(S*L) combine.

Per core:
  pass 1: x resident in SBUF; per 128-point tile: sum-of-squares accum
          (ACT), 1/norm per group, one-hot [128,64]*rr (DVE), one
          matmul accumulating the [64, 256] centroid sums in PSUM.
  mu:     scale by 1/counts, append w*WSCALE column -> [128, 257] f16
          table (rows 64..127 zero).
  push:   32 paired rotation matmuls on a [128, 256] double-stacked mu
          block (two rotation offsets per matmul via block-diagonal
          permutations).
  pass 2: per 4 tiles: seg broadcast matmul + one-hot transpose compare
          (DVE, from PSUM); per tile: one gather matmul -> per-point
          mu+w, diff (DVE), L1 accum (ACT); per group of G tiles:
          relu/square/weight + a ones-matmul partition reduction into
          per-tile pull sums.
All inputs are packed into one f16 plane + one f32 plane; outputs into
one f32 plane (q2 rotation distances + per-tile pull sums).
Host: loss = (sum(ts)/WSCALE + push)/N.
"""

import numpy as np

import concourse.bass as bass
import concourse.bacc as bacc
import concourse.mybir as mybir
import concourse.tile as tile

f32 = mybir.dt.float32
f16 = mybir.dt.float16

# Problem shape (hardcoded per contract).
N_TOTAL = 262144
D = 256
S = 8
L = 64
NCORES = 8
DELTA_V = 0.5
DELTA_D = 1.5
G = 16            # tiles per group
NPAIR = 32        # paired push iterations (covers k = 1..63)
WSCALE = 1024.0   # pull-weight scale to keep w in f16 normal range

AluOp = mybir.AluOpType
ActFn = mybir.ActivationFunctionType

# f16 plane column layout (after the x block of T_pad*D cols):
#   iota64 (64) | perms2 (NPAIR*128) | ones (128) | segrow (T_pad*128)
C16_CONST = 64 + NPAIR * 128 + 128


def build_nc(T_pad: int, reps: int = 1,
             phases: tuple = ("p1", "push", "p2")):
    """SPMD program for one core holding T_pad tiles of 128 points."""
    assert T_pad % G == 0
    NCH = 16                     # x load chunks
    CHT = T_pad // NCH
    assert CHT * NCH == T_pad
    BC = 4                       # tiles per seg-broadcast matmul
    XA = T_pad * D               # const block base in f16 plane
    SRB = XA + C16_CONST         # segrow base in f16 plane
    C16 = SRB + T_pad * 128
    C32 = T_pad + 4
    OC = NPAIR + T_pad

    nc = bacc.Bacc("TRN2", target_bir_lowering=False, debug=False,
                   num_devices=1)

    p16_in = nc.dram_tensor("p16", [128, C16], f16, kind="ExternalInput")
    p32_in = nc.dram_tensor("p32", [128, C32], f32, kind="ExternalInput")
    po_out = nc.dram_tensor("po", [128, OC], f32, kind="ExternalOutput")

    with tile.TileContext(nc) as tc:
        with (
            tc.tile_pool(name="const", bufs=1) as constp,
            tc.tile_pool(name="xres", bufs=NCH) as xp,
            tc.tile_pool(name="norm", bufs=1) as normp,
            tc.tile_pool(name="oh", bufs=4) as ohp,
            tc.tile_pool(name="srow", bufs=3) as srowp,
            tc.tile_pool(name="oht", bufs=8) as ohtp,
            tc.tile_pool(name="diff", bufs=4) as diffp,
            tc.tile_pool(name="sink", bufs=3) as sinkp,
            tc.tile_pool(name="mut", bufs=1) as mutp,
            tc.tile_pool(name="grp", bufs=2) as grpp,
            tc.tile_pool(name="small", bufs=2) as smallp,
        ):
            # ---- packed constants ----
            c16 = constp.tile([128, C16_CONST], f16)
            nc.sync.dma_start(c16[:], p16_in.ap()[:, XA:XA + C16_CONST])
            iota64_sb = c16[:, 0:64]
            perms2_sb = c16[:, 64:64 + NPAIR * 128]
            ones_sb = c16[0:1, 64 + NPAIR * 128:64 + NPAIR * 128 + 128]
            c32 = constp.tile([128, C32], f32)
            nc.sync.dma_start(c32[:], p32_in[:])
            segcol_sb = c32[:, 0:T_pad]
            iotap_sb = c32[:, T_pad:T_pad + 1]
            onesw_sb = c32[:, T_pad + 1:T_pad + 2]
            wblk_sb = c32[0:L, T_pad + 2:T_pad + 3]
            crecip_sb = c32[0:L, T_pad + 3:T_pad + 4]
            negdv_sb = constp.tile([128, 1], f32)
            nc.vector.memset(negdv_sb[:], -DELTA_V)
            eps_sb = constp.tile([128, 1], f32)
            nc.vector.memset(eps_sb[:], 1e-8)

            # ---- resident x (chunked so reads can start early) ----
            xch = []
            for i in range(NCH):
                c0, c1 = i * CHT, (i + 1) * CHT
                xt_ch = xp.tile([128, (c1 - c0) * D], f16, tag="xch",
                                name=f"xch{i}")
                nc.sync.dma_start(xt_ch[:], p16_in.ap()[:, c0 * D:c1 * D])
                xch.append(xt_ch)

            def xt(t):
                return xch[t // CHT][:, (t % CHT) * D:(t % CHT + 1) * D]

            for rep in range(reps):
                ss_all = normp.tile([128, T_pad], f32, tag="ss", name="ss")
                rr_all = normp.tile([128, T_pad], f32, tag="rr", name="rr")
                d1_all = normp.tile([128, T_pad], f32, tag="d1", name="d1")
                wc_all = normp.tile([128, T_pad], f32, tag="wc", name="wc")

                # ---- pass 1: local centroid sums of normalized points ----
                with tc.tile_pool(name="sumsps", bufs=1, space="PSUM") as sp:
                    ps_sums = sp.tile([L, D], f32, tag="sums", name="ps_sums")
                    for g in range(T_pad // G if "p1" in phases else 0):
                        t0 = g * G
                        for j in range(G):
                            t = t0 + j
                            sq_sink = sinkp.tile([128, D], f16, tag="sqsink")
                            nc.scalar.activation(
                                sq_sink[:], xt(t), ActFn.Square,
                                accum_out=ss_all[:, t:t + 1],
                            )
                        nn_g = grpp.tile([128, G], f32, tag="nn")
                        nc.scalar.activation(
                            nn_g[:], ss_all[:, t0:t0 + G],
                            ActFn.Sqrt, bias=eps_sb[:],
                        )
                        nc.vector.reciprocal(rr_all[:, t0:t0 + G], nn_g[:])
                        for j in range(G):
                            t = t0 + j
                            oh = ohp.tile([128, L], f16, tag="oh")
                            nc.vector.tensor_scalar(
                                oh[:], iota64_sb, segcol_sb[:, t:t + 1],
                                rr_all[:, t:t + 1],
                                op0=AluOp.is_equal, op1=AluOp.mult,
                            )
                            nc.tensor.matmul(
                                ps_sums[:], oh[:], xt(t),
                                start=(t == 0), stop=(t == T_pad - 1),
                            )

                    # ---- mu table: [128, 257] f16, rows 64.. zero ----
                    mut_h = mutp.tile([128, D + 1], f16, tag="mut",
                                      name="mut_h")
                    nc.vector.memset(mut_h[:], 0.0)
                    if "p1" in phases:
                        nc.vector.tensor_scalar(
                            mut_h[0:L, 0:D], ps_sums[:], crecip_sb, None,
                            op0=AluOp.mult,
                        )
                    nc.vector.tensor_copy(mut_h[0:L, D:D + 1], wblk_sb)

                # ---- push: paired rotation distances on own mu block ----
                q2_sb = smallp.tile([128, NPAIR], f32, tag="q", name="q2_sb")
                mua2 = mutp.tile([128, D], f16, tag="mua2", name="mua2")
                nc.vector.tensor_copy(mua2[0:L, :], mut_h[0:L, 0:D])
                nc.vector.tensor_copy(mua2[L:128, :], mut_h[0:L, 0:D])
                with (
                    tc.tile_pool(name="rotps", bufs=2, space="PSUM") as rotp,
                    tc.tile_pool(name="pdiff", bufs=3) as pdp,
                ):
                    for pi in range(NPAIR if "push" in phases else 0):
                        ps_rot = rotp.tile([128, D], f32, tag="rot")
                        nc.tensor.matmul(
                            ps_rot[:], perms2_sb[:, pi * 128:(pi + 1) * 128],
                            mua2[:], start=True, stop=True,
                        )
                        pdiff = pdp.tile([128, D], f32, tag="pdiff")
                        nc.vector.tensor_sub(pdiff[:], mua2[:], ps_rot[:])
                        psink = pdp.tile([128, D], f32, tag="psink")
                        nc.scalar.activation(
                            psink[:], pdiff[:], ActFn.Abs,
                            accum_out=q2_sb[:, pi:pi + 1],
                        )
                    if "push" not in phases:
                        nc.vector.memset(q2_sb[:], 0.0)
                nc.sync.dma_start(po_out.ap()[:, 0:NPAIR], q2_sb[:])

                # ---- pass 2: pull term ----
                with (
                    tc.tile_pool(name="bcps", bufs=2, space="PSUM") as bcp,
                    tc.tile_pool(name="mups", bufs=3, space="PSUM") as mup,
                    tc.tile_pool(name="tsps", bufs=1, space="PSUM") as tsp,
                ):
                    ps_ts = tsp.tile([1, T_pad], f32, tag="ts", name="ps_ts")
                    if "p2" not in phases:
                        nc.vector.memset(ps_ts[:], 0.0)
                    for g in range(T_pad // G if "p2" in phases else 0):
                        t0 = g * G
                        srow_g = srowp.tile([1, G * 128], f16, tag="srow")
                        nc.sync.dma_start(
                            srow_g[:],
                            p16_in.ap()[0:1, SRB + t0 * 128:
                                        SRB + (t0 + G) * 128],
                        )
                        oht4s = []
                        for b in range(G // BC):
                            ps_bc = bcp.tile([128, BC * 128], f32, tag="bc")
                            nc.tensor.matmul(
                                ps_bc[:], ones_sb,
                                srow_g[:, b * BC * 128:(b + 1) * BC * 128],
                                start=True, stop=True,
                            )
                            oht4 = ohtp.tile([128, BC * 128], f16, tag="oht")
                            nc.vector.tensor_scalar(
                                oht4[:], ps_bc[:], iotap_sb, None,
                                op0=AluOp.is_equal,
                            )
                            oht4s.append(oht4)
                        for j in range(G):
                            t = t0 + j
                            oht = oht4s[j // BC][:, (j % BC) * 128:
                                                 (j % BC + 1) * 128]
                            ps_mu = mup.tile([128, D + 1], f32, tag="mu")
                            nc.tensor.matmul(
                                ps_mu[:], oht, mut_h[:],
                                start=True, stop=True,
                            )
                            diff = diffp.tile([128, D], f32, tag="diff")
                            nc.vector.scalar_tensor_tensor(
                                diff[:], xt(t), rr_all[:, t:t + 1],
                                ps_mu[:, 0:D],
                                op0=AluOp.mult, op1=AluOp.subtract,
                            )
                            ab_sink = sinkp.tile([128, D], f32, tag="absink")
                            nc.scalar.activation(
                                ab_sink[:], diff[:], ActFn.Abs,
                                accum_out=d1_all[:, t:t + 1],
                            )
                            nc.vector.tensor_copy(
                                wc_all[:, t:t + 1], ps_mu[:, D:D + 1],
                            )
                        t1g = grpp.tile([128, G], f32, tag="t1g")
                        nc.scalar.activation(
                            t1g[:], d1_all[:, t0:t0 + G], ActFn.Relu,
                            bias=negdv_sb[:],
                        )
                        t2g = grpp.tile([128, G], f32, tag="t2g")
                        nc.vector.tensor_mul(t2g[:], t1g[:], t1g[:])
                        vg = grpp.tile([128, G], f32, tag="vg")
                        nc.vector.tensor_mul(vg[:], t2g[:],
                                             wc_all[:, t0:t0 + G])
                        nc.tensor.matmul(
                            ps_ts[0:1, t0:t0 + G], onesw_sb, vg[:],
                            start=True, stop=True,
                        )
                    ts_sb = smallp.tile([1, T_pad], f32, tag="tssb")
                    nc.vector.tensor_copy(ts_sb[:], ps_ts[:])
                    nc.sync.dma_start(
                        po_out.ap()[0:1, NPAIR:NPAIR + T_pad], ts_sb[:])

    nc.compile()
    return nc


def host_tables(labels: np.ndarray, subbatch: np.ndarray):
    """Everything derivable from the integer inputs alone."""
    seg = (subbatch.astype(np.int64) * L + labels.astype(np.int64)).astype(np.int32)
    counts = np.bincount(seg, minlength=S * L).astype(np.float64)
    present = counts > 0
    M = present.reshape(S, L).sum(axis=1).astype(np.float64)
    valid = M > 1.0
    M_per_seg = np.repeat(M, L)
    valid_per_seg = np.repeat(valid, L)
    w = np.where(
        valid_per_seg, 1.0 / (M_per_seg * np.maximum(counts, 1.0)), 0.0
    ).astype(np.float32)
    crecip = (1.0 / np.maximum(counts, 1.0)).astype(np.float32)
    return seg, counts, present, M, valid, w, crecip


def pick_tpad(subbatch: np.ndarray) -> int:
    counts_sb = np.bincount(subbatch, minlength=S)
    T = int(np.ceil(counts_sb.max() / 128))
    return ((T + G - 1) // G) * G


def make_in_maps(outputs: np.ndarray, labels: np.ndarray,
                 subbatch: np.ndarray, T_pad: int | None = None):
    n = outputs.shape[0]
    tables = host_tables(labels, subbatch)
    seg, counts, present, M, valid, w, crecip = tables
    if T_pad is None:
        T_pad = pick_tpad(subbatch)
    n_pad = T_pad * 128
    XA = T_pad * D
    SRB = XA + C16_CONST
    C16 = SRB + T_pad * 128
    C32 = T_pad + 4

    order = np.argsort(subbatch, kind="stable")
    counts_sb = np.bincount(subbatch, minlength=S)
    offs = np.concatenate([[0], np.cumsum(counts_sb)])

    xh = outputs.astype(np.float16)
    labf = labels.astype(np.float32)

    # paired block-diagonal rotation permutations [128, NPAIR, 128]
    perms2 = np.zeros((128, NPAIR, 128), np.float16)
    pr, mr = np.meshgrid(np.arange(L), np.arange(L), indexing="ij")
    for pi in range(NPAIR):
        k1 = 2 * pi + 1
        perms2[0:L, pi, 0:L] = (pr == (mr + k1) % L)
        if 2 * pi + 2 < L:
            k2 = 2 * pi + 2
            perms2[L:128, pi, L:128] = (pr == (mr + k2) % L)
    perms2 = perms2.reshape(128, NPAIR * 128)

    in_maps = []
    for c in range(NCORES):
        idx = order[offs[c]:offs[c + 1]]
        cnt = idx.size
        xc = np.zeros((n_pad, D), np.float16)
        xc[:cnt] = xh[idx]
        xc = np.ascontiguousarray(
            xc.reshape(T_pad, 128, D).transpose(1, 0, 2)
        ).reshape(128, T_pad * D)
        segl = np.full((n_pad,), -1.0, np.float32)
        segl[:cnt] = labf[idx]

        p16 = np.zeros((128, C16), np.float16)
        p16[:, 0:XA] = xc
        p16[:, XA:XA + 64] = np.arange(L, dtype=np.float16)[None, :]
        p16[:, XA + 64:XA + 64 + NPAIR * 128] = perms2
        p16[:, XA + 64 + NPAIR * 128:SRB] = 1.0
        p16[0, SRB:] = segl.astype(np.float16)

        blk = slice(c * L, (c + 1) * L)
        p32 = np.zeros((128, C32), np.float32)
        p32[:, 0:T_pad] = segl.reshape(T_pad, 128).T
        p32[:, T_pad] = np.arange(128, dtype=np.float32)
        p32[:, T_pad + 1] = 1.0
        p32[0:L, T_pad + 2] = w[blk] * WSCALE
        p32[0:L, T_pad + 3] = crecip[blk]
        in_maps.append({"p16": p16, "p32": p32})
    return in_maps, tables, T_pad


def combine(results, tables, n: int):
    """Host combine of the per-core outputs into the scalar loss."""
    seg, counts, present, M, valid, w, crecip = tables
    T_pad = results[0]["po"].shape[1] - NPAIR
    pull_total = np.float64(0.0)
    for r in results:
        pull_total += r["po"][0, NPAIR:].astype(np.float64).sum() / WSCALE

    push_total = np.float64(0.0)
    pres_sl = present.reshape(S, L)
    a = np.arange(L)
    for sb in range(S):
        if not valid[sb]:
            continue
        q2 = results[sb]["po"][:, 0:NPAIR].astype(np.float64)  # [128, NPAIR]
        dist = np.zeros((L, L))
        for pi in range(NPAIR):
            k1 = 2 * pi + 1
            dist[a, (a + k1) % L] = q2[0:L, pi]
            if 2 * pi + 2 < L:
                k2 = 2 * pi + 2
                dist[a, (a + k2) % L] = q2[L:128, pi]
        p = pres_sl[sb]
        mask = p[:, None] & p[None, :] & ~np.eye(L, dtype=bool)
        r = np.maximum(2.0 * DELTA_D - dist, 0.0) ** 2
        push = np.where(mask, r, 0.0).sum()
        push_total += push / max(M[sb] * (M[sb] - 1.0), 1.0)

    return np.float32((pull_total + push_total) / n)


_NC_CACHE: dict = {}


def _get_nc(T_pad: int):
    if T_pad not in _NC_CACHE:
        _NC_CACHE[T_pad] = build_nc(T_pad)
    return _NC_CACHE[T_pad]


def kernel(outputs, labels, subbatch_indices):
    from concourse.bass_utils import run_bass_kernel_spmd

    outputs = np.asarray(outputs, dtype=np.float32)
    labels = np.asarray(labels, dtype=np.int32)
    subbatch_indices = np.asarray(subbatch_indices, dtype=np.int32)
    n = outputs.shape[0]

    in_maps, tables, T_pad = make_in_maps(outputs, labels, subbatch_indices)
    nc = _get_nc(T_pad)
    res = run_bass_kernel_spmd(nc, in_maps, list(range(NCORES)))
    return combine(res.results, tables, n)
